# revision 4
# baseline (speedup 1.0000x reference)
"""Distributed multi-head attention kernel for 8 TRN2 NeuronCores.

Problem: x[4, 2048, 1024] @ w_qkv[1024, 3072] -> qkv -> 16-head attention
         -> out[4, 2048, 1024], fp32.

Sharding (data parallel batch x tensor parallel heads):
  core c handles batch b = c // 2 and heads h0 = (c % 2) * 8 .. h0 + 8.
  Each core receives x_b [2048, 1024] and the w_qkv column slice for its
  8 heads ([1024, 1536] = q|k|v each 512 cols), produces out[b, :, 512-slice].
  All 64 (batch, head) attention problems are independent -> no collectives.

Per-core kernel (all matmuls bf16 with fp32 PSUM accumulation):
  1. x -> bf16 -> PE-transpose -> xT [c, i]       (contraction dim on partitions)
  2. qkT = w_qk.T @ x.T (per f-tile), v = x @ w_v (natural layout)
     v is stored with a fused ones-column per head: v'[:, h] = [v_h | 1]
  3. per head: dotsT[j, i] = kT.T @ qT; P = exp(dots * 0.125) (ScalarE,
     scale fused); U'[d|Z, i] += v'_h.T @ P  (PV matmul; row 64 = softmax
     denominator Z via the ones column)
  4. PE-transpose U' -> [i, d|Z]; out = U * (1/Z) (DVE); DMA out.
"""

import numpy as np

B, N, DIM = 4, 2048, 1024
HEADS, DIM_HEAD = 16, 64
INNER = HEADS * DIM_HEAD
HPC = 8                 # heads per core
FQ = HPC * DIM_HEAD     # 512 = per-core q/k/v column count
NCORES = 8

P = 128
CT = DIM // P           # 8 c-tiles (contraction)
IT = N // P             # 16 i-tiles
JT = N // P             # 16 j-tiles
NIH = 2                 # i halves
IH = N // NIH           # 1024

_CACHE = {}


def _build():
    import concourse.bass as bass
    import concourse.mybir as mybir
    import concourse.tile as tile
    from concourse import bacc
    from concourse.masks import make_identity

    f32 = mybir.dt.float32
    bf16 = mybir.dt.bfloat16
    Exp = mybir.ActivationFunctionType.Exp

    nc = bacc.Bacc(None, target_bir_lowering=False)
    x_d = nc.dram_tensor("x", [N, DIM], f32, kind="ExternalInput")
    w_d = nc.dram_tensor("w", [DIM, 3 * FQ], f32, kind="ExternalInput")
    o_d = nc.dram_tensor("o", [N, FQ], f32, kind="ExternalOutput")

    with tile.TileContext(nc) as tc, \
         tc.tile_pool(name="persist", bufs=1) as persist:
        ident = persist.tile([P, P], bf16, tag="ident", name="ident")
        make_identity(nc, ident[:])

        # persistent buffers
        xT = persist.tile([P, CT, N], bf16, tag="xT", name="xT")
        wsb = persist.tile([P, CT, 3 * FQ], bf16, tag="wsb", name="wsb")
        qkT = persist.tile([P, CT, N], bf16, tag="qkT", name="qkT")
        vp = persist.tile([P, JT, HPC * 65], bf16, tag="vp", name="vp")

        vp_heads = vp[:].rearrange("p j (h c) -> p j h c", c=65)
        nc.vector.memset(vp_heads[:, :, :, 64:65], 1.0)

        # ---- phase 1: load x, cast, transpose ----
        with tc.tile_pool(name="xload", bufs=3) as xload, \
             tc.tile_pool(name="xcast", bufs=3) as xcast, \
             tc.tile_pool(name="tps_x", bufs=4, space="PSUM") as tps_x:
            for it in range(IT):
                xf = xload.tile([P, DIM], f32)
                nc.sync.dma_start(xf[:], x_d[bass.ds(it * P, P), :])
                xb = xcast.tile([P, DIM], bf16)
                nc.vector.tensor_copy(xb[:], xf[:])
                for ct in range(CT):
                    tp = tps_x.tile([P, P], bf16)
                    nc.tensor.transpose(tp[:], xb[:, bass.ds(ct * P, P)], ident[:])
                    nc.vector.tensor_copy(xT[:, ct, bass.ds(it * P, P)], tp[:])

        # ---- phase 2: load w, cast ----
        with tc.tile_pool(name="wload", bufs=2) as wload:
            for ct in range(CT):
                wf = wload.tile([P, 3 * FQ], f32)
                nc.sync.dma_start(wf[:], w_d[bass.ds(ct * P, P), :])
                nc.vector.tensor_copy(wsb[:, ct], wf[:])

        # ---- phase 3: QKV ----
        with tc.tile_pool(name="qkvp", bufs=2, space="PSUM") as qkvp:
            # qkT[f, i] = sum_c w[c, f] * xT[c, i]   (f < 1024: q then k)
            for ft in range(CT):
                for ic in range(4):
                    ps = qkvp.tile([P, 512], f32)
                    for ct in range(CT):
                        nc.tensor.matmul(
                            ps[:],
                            wsb[:, ct, bass.ds(ft * P, P)],
                            xT[:, ct, bass.ds(ic * 512, 512)],
                            start=(ct == 0), stop=(ct == CT - 1),
                        )
                    nc.vector.tensor_copy(qkT[:, ft, bass.ds(ic * 512, 512)], ps[:])
            # v[i, fv] = sum_c xT[c, i] * w_v[c, fv] ; scatter into v' (65-stride)
            for it in range(IT):
                ps = qkvp.tile([P, 512], f32, tag="ps")
                for ct in range(CT):
                    nc.tensor.matmul(
                        ps[:],
                        xT[:, ct, bass.ds(it * P, P)],
                        wsb[:, ct, bass.ds(2 * FQ, FQ)],
                        start=(ct == 0), stop=(ct == CT - 1),
                    )
                nc.vector.tensor_copy(
                    vp_heads[:, it, :, 0:64],
                    ps[:].rearrange("p (h c) -> p h c", c=64),
                )

        # ---- phase 4: attention ----
        with tc.tile_pool(name="dotsp", bufs=2, space="PSUM") as dotsp, \
             tc.tile_pool(name="upp", bufs=1, space="PSUM") as upp, \
             tc.tile_pool(name="tpp", bufs=2, space="PSUM") as tpp, \
             tc.tile_pool(name="ptp", bufs=3) as ptp, \
             tc.tile_pool(name="uep", bufs=2) as uep, \
             tc.tile_pool(name="recp", bufs=2) as recp, \
             tc.tile_pool(name="normp", bufs=4) as normp:
            for h in range(HPC):
                pb = (h % 2) * 64          # partition base of this head's d-slice
                qft = h // 2               # q f-tile
                kft = 4 + h // 2           # k f-tile
                for ih in range(NIH):
                    ups = upp.tile([65, IH], f32)
                    for j in range(JT):
                        dt_ = dotsp.tile([P, IH], f32)
                        for half in range(2):
                            nc.tensor.matmul(
                                dt_[:, bass.ds(half * 512, 512)],
                                qkT[bass.ds(pb, 64), kft, bass.ds(j * P, P)],
                                qkT[bass.ds(pb, 64), qft,
                                    bass.ds(ih * IH + half * 512, 512)],
                                start=True, stop=True,
                            )
                        pt = ptp.tile([P, IH], bf16)
                        nc.scalar.activation(pt[:], dt_[:], Exp, scale=0.125)
                        for half in range(2):
                            nc.tensor.matmul(
                                ups[:, bass.ds(half * 512, 512)],
                                vp_heads[:, j, h, :],
                                pt[:, bass.ds(half * 512, 512)],
                                start=(j == 0), stop=(j == JT - 1),
                            )
                    ue = uep.tile([65, IH], bf16)
                    nc.vector.tensor_copy(ue[:], ups[:])
                    for b in range(IH // P):
                        tp = tpp.tile([P, 65], bf16)
                        nc.tensor.transpose(
                            tp[:], ue[:, bass.ds(b * P, P)], ident[0:65, 0:65]
                        )
                        rec = recp.tile([P, 1], f32)
                        nc.vector.reciprocal(rec[:], tp[:, 64:65])
                        nrm = normp.tile([P, 64], f32)
                        nc.vector.tensor_scalar_mul(nrm[:], tp[:, 0:64], rec[:])
                        it = ih * (IH // P) + b
                        nc.sync.dma_start(
                            o_d[bass.ds(it * P, P), bass.ds(h * 64, 64)], nrm[:]
                        )
    nc.finalize()
    return nc


def _get_nc():
    if "nc" not in _CACHE:
        _CACHE["nc"] = _build()
    return _CACHE["nc"]


def kernel(x: np.ndarray, w_qkv: np.ndarray) -> np.ndarray:
    from concourse.bass_utils import run_bass_kernel_spmd

    x = np.asarray(x, dtype=np.float32)
    w_qkv = np.asarray(w_qkv, dtype=np.float32)

    in_maps = []
    for c in range(NCORES):
        b, hh = c // 2, c % 2
        qo = hh * FQ
        ws = np.concatenate(
            [w_qkv[:, qo:qo + FQ],
             w_qkv[:, INNER + qo:INNER + qo + FQ],
             w_qkv[:, 2 * INNER + qo:2 * INNER + qo + FQ]], axis=1)
        in_maps.append({
            "x": np.ascontiguousarray(x[b]),
            "w": np.ascontiguousarray(ws),
        })

    nc = _get_nc()
    res = run_bass_kernel_spmd(nc, in_maps, core_ids=list(range(NCORES)))

    out = np.empty((B, N, INNER), np.float32)
    for c in range(NCORES):
        b, hh = c // 2, c % 2
        out[b, :, hh * FQ:(hh + 1) * FQ] = res.results[c]["o"]
    return out


# revision 8
# speedup vs baseline: 1.1940x; 1.1940x over previous
"""Distributed multi-head attention kernel for 8 TRN2 NeuronCores.

Problem: x[4, 2048, 1024] @ w_qkv[1024, 3072] -> qkv -> 16-head attention
         -> out[4, 2048, 1024], fp32.

Sharding (data parallel batch x tensor parallel heads):
  core c handles batch b = c // 2 and heads h0 = (c % 2) * 8 .. h0 + 8.
  Each core receives x_b [2048, 1024] and the w_qkv column slice for its
  8 heads ([1024, 1536] = q|k|v each 512 cols), produces out[b, :, 512-slice].
  All 64 (batch, head) attention problems are independent -> no collectives.

Per-core kernel (all matmuls bf16 with fp32 PSUM accumulation):
  1. x -> bf16 -> PE-transpose -> xT [c, i]       (contraction dim on partitions)
  2. qkT = w_qk.T @ x.T (per f-tile), v = x @ w_v (natural layout)
     v is stored with a fused ones-column per head: v'[:, h] = [v_h | 1]
  3. per head: dotsT[j, i] = kT.T @ qT; P = exp(dots * 0.125) (ScalarE,
     scale fused); U'[d|Z, i] += v'_h.T @ P  (PV matmul; row 64 = softmax
     denominator Z via the ones column)
  4. PE-transpose U' -> [i, d|Z]; out = U * (1/Z) (DVE); DMA out.
"""

import numpy as np

B, N, DIM = 4, 2048, 1024
HEADS, DIM_HEAD = 16, 64
INNER = HEADS * DIM_HEAD
HPC = 8                 # heads per core
FQ = HPC * DIM_HEAD     # 512 = per-core q/k/v column count
NCORES = 8

P = 128
CT = DIM // P           # 8 c-tiles (contraction)
IT = N // P             # 16 i-tiles
JT = N // P             # 16 j-tiles
NIH = 2                 # i halves
IH = N // NIH           # 1024

_CACHE = {}


def _build():
    import concourse.bass as bass
    import concourse.mybir as mybir
    import concourse.tile as tile
    from concourse import bacc
    from concourse.masks import make_identity

    f32 = mybir.dt.float32
    bf16 = mybir.dt.bfloat16
    Exp = mybir.ActivationFunctionType.Exp

    nc = bacc.Bacc(None, target_bir_lowering=False)
    x_d = nc.dram_tensor("x", [N, DIM], f32, kind="ExternalInput")
    w_d = nc.dram_tensor("w", [DIM, 3 * FQ], f32, kind="ExternalInput")
    o_d = nc.dram_tensor("o", [N, FQ], f32, kind="ExternalOutput")

    with tile.TileContext(nc) as tc, \
         tc.tile_pool(name="persist", bufs=1) as persist:
        ident = persist.tile([P, P], bf16, tag="ident", name="ident")
        make_identity(nc, ident[:])

        # persistent buffers
        xT = persist.tile([P, CT, N], bf16, tag="xT", name="xT")
        wsb = persist.tile([P, CT, 3 * FQ], bf16, tag="wsb", name="wsb")
        qkT = persist.tile([P, CT, N], bf16, tag="qkT", name="qkT")
        vp = persist.tile([P, JT, HPC * 65], bf16, tag="vp", name="vp")

        vp_heads = vp[:].rearrange("p j (h c) -> p j h c", c=65)
        nc.vector.memset(vp_heads[:, :, :, 64:65], 1.0)

        # ---- phase 1: load x, cast, transpose ----
        with tc.tile_pool(name="xload", bufs=3) as xload, \
             tc.tile_pool(name="xcast", bufs=3) as xcast, \
             tc.tile_pool(name="tps_x", bufs=4, space="PSUM") as tps_x:
            for it in range(IT):
                xf = xload.tile([P, DIM], f32)
                nc.sync.dma_start(xf[:], x_d[bass.ds(it * P, P), :])
                xb = xcast.tile([P, DIM], bf16)
                nc.vector.tensor_copy(xb[:], xf[:])
                for ct in range(CT):
                    tp = tps_x.tile([P, P], bf16)
                    nc.tensor.transpose(tp[:], xb[:, bass.ds(ct * P, P)], ident[:])
                    nc.vector.tensor_copy(xT[:, ct, bass.ds(it * P, P)], tp[:])

        # ---- phase 2: load w, cast ----
        with tc.tile_pool(name="wload", bufs=2) as wload:
            for ct in range(CT):
                wf = wload.tile([P, 3 * FQ], f32)
                nc.sync.dma_start(wf[:], w_d[bass.ds(ct * P, P), :])
                nc.vector.tensor_copy(wsb[:, ct], wf[:])

        # ---- phases 3+4 interleaved: QKV for head-pair hp emitted right
        # before the attention of heads 2hp, 2hp+1, so the PE fills ACT-bound
        # gaps with the next pair's QKV matmuls (keeps HAM warm, hides QKV) --
        with tc.tile_pool(name="qkvp", bufs=1, space="PSUM") as qkvp, \
             tc.tile_pool(name="dotsp", bufs=2, space="PSUM") as dotsp, \
             tc.tile_pool(name="upp", bufs=1, space="PSUM") as upp, \
             tc.tile_pool(name="tpp", bufs=1, space="PSUM") as tpp, \
             tc.tile_pool(name="ptp", bufs=3) as ptp, \
             tc.tile_pool(name="uep", bufs=2) as uep, \
             tc.tile_pool(name="recp", bufs=2) as recp, \
             tc.tile_pool(name="normp", bufs=4) as normp:

            def emit_qkv_for_pair(hp):
                # qkT f-tiles: q -> ft=hp, k -> ft=4+hp (cols hp*128..+128)
                for ft in (hp, 4 + hp):
                    for ic in range(4):
                        ps = qkvp.tile([P, 512], f32, tag="qkv")
                        for ct in range(CT):
                            nc.tensor.matmul(
                                ps[:],
                                wsb[:, ct, bass.ds(ft * P, P)],
                                xT[:, ct, bass.ds(ic * 512, 512)],
                                start=(ct == 0), stop=(ct == CT - 1),
                            )
                        nc.vector.tensor_copy(
                            qkT[:, ft, bass.ds(ic * 512, 512)], ps[:])
                # v (natural layout) for this pair: fv = hp*128 .. +128
                for it in range(IT):
                    ps = qkvp.tile([P, P], f32, tag="qkv")
                    for ct in range(CT):
                        nc.tensor.matmul(
                            ps[:],
                            xT[:, ct, bass.ds(it * P, P)],
                            wsb[:, ct, bass.ds(2 * FQ + hp * P, P)],
                            start=(ct == 0), stop=(ct == CT - 1),
                        )
                    nc.vector.tensor_copy(
                        vp_heads[:, it, bass.ds(2 * hp, 2), 0:64],
                        ps[:].rearrange("p (h c) -> p h c", c=64),
                    )

            def emit_attention_head(h):
                pb = (h % 2) * 64          # partition base of this head's d-slice
                qft = h // 2               # q f-tile
                kft = 4 + h // 2           # k f-tile
                for ih in range(NIH):
                    ups = upp.tile([65, IH], f32, tag="ups")
                    for j in range(JT):
                        dt_ = dotsp.tile([P, IH], f32, tag="dt")
                        for half in range(2):
                            nc.tensor.matmul(
                                dt_[:, bass.ds(half * 512, 512)],
                                qkT[bass.ds(pb, 64), kft, bass.ds(j * P, P)],
                                qkT[bass.ds(pb, 64), qft,
                                    bass.ds(ih * IH + half * 512, 512)],
                                start=True, stop=True,
                            )
                        pt = ptp.tile([P, IH], bf16, tag="pt")
                        nc.scalar.activation(pt[:], dt_[:], Exp, scale=0.125)
                        for half in range(2):
                            nc.tensor.matmul(
                                ups[:, bass.ds(half * 512, 512)],
                                vp_heads[:, j, h, :],
                                pt[:, bass.ds(half * 512, 512)],
                                start=(j == 0), stop=(j == JT - 1),
                            )
                    ue = uep.tile([65, IH], bf16, tag="ue")
                    nc.vector.tensor_copy(ue[:], ups[:])
                    for b in range(IH // P):
                        tp = tpp.tile([P, 65], bf16, tag="tp")
                        nc.tensor.transpose(
                            tp[:], ue[:, bass.ds(b * P, P)], ident[0:65, 0:65]
                        )
                        rec = recp.tile([P, 1], f32)
                        nc.vector.reciprocal(rec[:], tp[:, 64:65])
                        nrm = normp.tile([P, 64], f32)
                        nc.vector.tensor_scalar_mul(nrm[:], tp[:, 0:64], rec[:])
                        it = ih * (IH // P) + b
                        nc.sync.dma_start(
                            o_d[bass.ds(it * P, P), bass.ds(h * 64, 64)], nrm[:]
                        )

            for hp in range(4):
                emit_qkv_for_pair(hp)
                emit_attention_head(2 * hp)
                emit_attention_head(2 * hp + 1)
    nc.finalize()
    return nc


def _get_nc():
    if "nc" not in _CACHE:
        _CACHE["nc"] = _build()
    return _CACHE["nc"]


def kernel(x: np.ndarray, w_qkv: np.ndarray) -> np.ndarray:
    from concourse.bass_utils import run_bass_kernel_spmd

    x = np.asarray(x, dtype=np.float32)
    w_qkv = np.asarray(w_qkv, dtype=np.float32)

    in_maps = []
    for c in range(NCORES):
        b, hh = c // 2, c % 2
        qo = hh * FQ
        ws = np.concatenate(
            [w_qkv[:, qo:qo + FQ],
             w_qkv[:, INNER + qo:INNER + qo + FQ],
             w_qkv[:, 2 * INNER + qo:2 * INNER + qo + FQ]], axis=1)
        in_maps.append({
            "x": np.ascontiguousarray(x[b]),
            "w": np.ascontiguousarray(ws),
        })

    nc = _get_nc()
    res = run_bass_kernel_spmd(nc, in_maps, core_ids=list(range(NCORES)))

    out = np.empty((B, N, INNER), np.float32)
    for c in range(NCORES):
        b, hh = c // 2, c % 2
        out[b, :, hh * FQ:(hh + 1) * FQ] = res.results[c]["o"]
    return out


# revision 9
# speedup vs baseline: 1.2881x; 1.0788x over previous
"""Distributed multi-head attention kernel for 8 TRN2 NeuronCores.

Problem: x[4, 2048, 1024] @ w_qkv[1024, 3072] -> qkv -> 16-head attention
         -> out[4, 2048, 1024], fp32.

Sharding (data parallel batch x tensor parallel heads):
  core c handles batch b = c // 2 and heads h0 = (c % 2) * 8 .. h0 + 8.
  Each core receives x_b [2048, 1024] and the w_qkv column slice for its
  8 heads ([1024, 1536] = q|k|v each 512 cols), produces out[b, :, 512-slice].
  All 64 (batch, head) attention problems are independent -> no collectives.

Per-core kernel (all matmuls bf16 with fp32 PSUM accumulation):
  1. x -> bf16 -> PE-transpose -> xT [c, i]       (contraction dim on partitions)
  2. qkT = w_qk.T @ x.T (per f-tile), v = x @ w_v (natural layout)
     v is stored with a fused ones-column per head: v'[:, h] = [v_h | 1]
  3. per head: dotsT[j, i] = kT.T @ qT; P = exp(dots * 0.125) (ScalarE,
     scale fused); U'[d|Z, i] += v'_h.T @ P  (PV matmul; row 64 = softmax
     denominator Z via the ones column)
  4. PE-transpose U' -> [i, d|Z]; out = U * (1/Z) (DVE); DMA out.
"""

import numpy as np

B, N, DIM = 4, 2048, 1024
HEADS, DIM_HEAD = 16, 64
INNER = HEADS * DIM_HEAD
HPC = 8                 # heads per core
FQ = HPC * DIM_HEAD     # 512 = per-core q/k/v column count
NCORES = 8

P = 128
CT = DIM // P           # 8 c-tiles (contraction)
IT = N // P             # 16 i-tiles
JT = N // P             # 16 j-tiles
NIH = 2                 # i halves
IH = N // NIH           # 1024

_CACHE = {}


def _build():
    import concourse.bass as bass
    import concourse.mybir as mybir
    import concourse.tile as tile
    from concourse import bacc
    from concourse.masks import make_identity

    f32 = mybir.dt.float32
    bf16 = mybir.dt.bfloat16
    Exp = mybir.ActivationFunctionType.Exp

    nc = bacc.Bacc(None, target_bir_lowering=False)
    x_d = nc.dram_tensor("x", [N, DIM], f32, kind="ExternalInput")
    w_d = nc.dram_tensor("w", [DIM, 3 * FQ], f32, kind="ExternalInput")
    o_d = nc.dram_tensor("o", [N, FQ], f32, kind="ExternalOutput")

    with tile.TileContext(nc) as tc, \
         tc.tile_pool(name="persist", bufs=1) as persist:
        ident = persist.tile([P, P], bf16, tag="ident", name="ident")
        make_identity(nc, ident[:])

        # persistent buffers
        xT = persist.tile([P, CT, N], bf16, tag="xT", name="xT")
        wsb = persist.tile([P, CT, 3 * FQ], bf16, tag="wsb", name="wsb")
        qkT = persist.tile([P, CT, N], bf16, tag="qkT", name="qkT")
        vp = persist.tile([P, JT, HPC * 65], bf16, tag="vp", name="vp")

        vp_heads = vp[:].rearrange("p j (h c) -> p j h c", c=65)
        nc.vector.memset(vp_heads[:, :, :, 64:65], 1.0)

        # ---- phase 1: load x, cast, transpose ----
        with tc.tile_pool(name="xload", bufs=3) as xload, \
             tc.tile_pool(name="xcast", bufs=3) as xcast, \
             tc.tile_pool(name="tps_x", bufs=4, space="PSUM") as tps_x:
            for it in range(IT):
                xf = xload.tile([P, DIM], f32)
                nc.sync.dma_start(xf[:], x_d[bass.ds(it * P, P), :])
                xb = xcast.tile([P, DIM], bf16)
                nc.vector.tensor_copy(xb[:], xf[:])
                for ct in range(CT):
                    tp = tps_x.tile([P, P], bf16)
                    nc.tensor.transpose(tp[:], xb[:, bass.ds(ct * P, P)], ident[:])
                    nc.vector.tensor_copy(xT[:, ct, bass.ds(it * P, P)], tp[:])

        # ---- phase 2: load w, cast ----
        with tc.tile_pool(name="wload", bufs=2) as wload:
            for ct in range(CT):
                wf = wload.tile([P, 3 * FQ], f32)
                nc.sync.dma_start(wf[:], w_d[bass.ds(ct * P, P), :])
                nc.vector.tensor_copy(wsb[:, ct], wf[:])

        # ---- phases 3+4 interleaved: QKV for head-pair hp emitted right
        # before the attention of heads 2hp, 2hp+1, so the PE fills ACT-bound
        # gaps with the next pair's QKV matmuls (keeps HAM warm, hides QKV) --
        with tc.tile_pool(name="qkvp", bufs=1, space="PSUM") as qkvp, \
             tc.tile_pool(name="dotsp", bufs=2, space="PSUM") as dotsp, \
             tc.tile_pool(name="upp", bufs=1, space="PSUM") as upp, \
             tc.tile_pool(name="tpp", bufs=1, space="PSUM") as tpp, \
             tc.tile_pool(name="ptp", bufs=3) as ptp, \
             tc.tile_pool(name="uep", bufs=2) as uep, \
             tc.tile_pool(name="recp", bufs=2) as recp, \
             tc.tile_pool(name="normp", bufs=4) as normp:

            def emit_qkv_for_pair(hp):
                # qkT f-tiles: q -> ft=hp, k -> ft=4+hp (cols hp*128..+128)
                for ft in (hp, 4 + hp):
                    for ic in range(4):
                        ps = qkvp.tile([P, 512], f32, tag="qkv")
                        for ct in range(CT):
                            nc.tensor.matmul(
                                ps[:],
                                wsb[:, ct, bass.ds(ft * P, P)],
                                xT[:, ct, bass.ds(ic * 512, 512)],
                                start=(ct == 0), stop=(ct == CT - 1),
                            )
                        nc.vector.tensor_copy(
                            qkT[:, ft, bass.ds(ic * 512, 512)], ps[:])
                # v (natural layout) for this pair: fv = hp*128 .. +128
                for it in range(IT):
                    ps = qkvp.tile([P, P], f32, tag="qkv")
                    for ct in range(CT):
                        nc.tensor.matmul(
                            ps[:],
                            xT[:, ct, bass.ds(it * P, P)],
                            wsb[:, ct, bass.ds(2 * FQ + hp * P, P)],
                            start=(ct == 0), stop=(ct == CT - 1),
                        )
                    nc.vector.tensor_copy(
                        vp_heads[:, it, bass.ds(2 * hp, 2), 0:64],
                        ps[:].rearrange("p (h c) -> p h c", c=64),
                    )

            def emit_epilogue(ups, ic, h):
                # U'[d|Z, 512] -> transpose 128-blocks -> normalize -> DMA out
                ue = uep.tile([65, 512], bf16, tag="ue")
                nc.vector.tensor_copy(ue[:], ups[:])
                for b in range(4):
                    tp = tpp.tile([P, 65], bf16, tag="tp")
                    nc.tensor.transpose(
                        tp[:], ue[:, bass.ds(b * P, P)], ident[0:65, 0:65]
                    )
                    rec = recp.tile([P, 1], f32)
                    nc.vector.reciprocal(rec[:], tp[:, 64:65])
                    nrm = normp.tile([P, 64], f32)
                    nc.vector.tensor_scalar_mul(nrm[:], tp[:, 0:64], rec[:])
                    it = ic * 4 + b
                    nc.sync.dma_start(
                        o_d[bass.ds(it * P, P), bass.ds(h * 64, 64)], nrm[:]
                    )

            def emit_attention_pair(hp):
                # heads hA = 2hp (partitions 0:64), hB = 2hp+1 (64:128) share
                # f-tiles qft/kft; dots for both packed into one [128, 1024]
                # psum (column halves) -> one exp ACTIVATE covers both.
                hA, hB = 2 * hp, 2 * hp + 1
                qft, kft = hp, 4 + hp
                for ic in range(4):          # i-chunks of 512
                    upsA = upp.tile([65, 512], f32, tag="upsA")
                    upsB = upp.tile([65, 512], f32, tag="upsB")
                    prev_pt = None
                    for j in range(JT):
                        dt_ = dotsp.tile([P, 1024], f32, tag="dt")
                        nc.tensor.matmul(
                            dt_[:, 0:512],
                            qkT[0:64, kft, bass.ds(j * P, P)],
                            qkT[0:64, qft, bass.ds(ic * 512, 512)],
                            start=True, stop=True,
                        )
                        nc.tensor.matmul(
                            dt_[:, 512:1024],
                            qkT[64:128, kft, bass.ds(j * P, P)],
                            qkT[64:128, qft, bass.ds(ic * 512, 512)],
                            start=True, stop=True,
                        )
                        pt = ptp.tile([P, 1024], bf16, tag="pt")
                        nc.scalar.activation(pt[:], dt_[:], Exp, scale=0.125)
                        if prev_pt is not None:
                            jj = j - 1
                            nc.tensor.matmul(
                                upsA[:], vp_heads[:, jj, hA, :],
                                prev_pt[:, 0:512],
                                start=(jj == 0), stop=False,
                            )
                            nc.tensor.matmul(
                                upsB[:], vp_heads[:, jj, hB, :],
                                prev_pt[:, 512:1024],
                                start=(jj == 0), stop=False,
                            )
                        prev_pt = pt
                    jj = JT - 1
                    nc.tensor.matmul(
                        upsA[:], vp_heads[:, jj, hA, :], prev_pt[:, 0:512],
                        start=False, stop=True,
                    )
                    nc.tensor.matmul(
                        upsB[:], vp_heads[:, jj, hB, :], prev_pt[:, 512:1024],
                        start=False, stop=True,
                    )
                    emit_epilogue(upsA, ic, hA)
                    emit_epilogue(upsB, ic, hB)

            for hp in range(4):
                emit_qkv_for_pair(hp)
                emit_attention_pair(hp)
    nc.finalize()
    return nc


def _get_nc():
    if "nc" not in _CACHE:
        _CACHE["nc"] = _build()
    return _CACHE["nc"]


def kernel(x: np.ndarray, w_qkv: np.ndarray) -> np.ndarray:
    from concourse.bass_utils import run_bass_kernel_spmd

    x = np.asarray(x, dtype=np.float32)
    w_qkv = np.asarray(w_qkv, dtype=np.float32)

    in_maps = []
    for c in range(NCORES):
        b, hh = c // 2, c % 2
        qo = hh * FQ
        ws = np.concatenate(
            [w_qkv[:, qo:qo + FQ],
             w_qkv[:, INNER + qo:INNER + qo + FQ],
             w_qkv[:, 2 * INNER + qo:2 * INNER + qo + FQ]], axis=1)
        in_maps.append({
            "x": np.ascontiguousarray(x[b]),
            "w": np.ascontiguousarray(ws),
        })

    nc = _get_nc()
    res = run_bass_kernel_spmd(nc, in_maps, core_ids=list(range(NCORES)))

    out = np.empty((B, N, INNER), np.float32)
    for c in range(NCORES):
        b, hh = c // 2, c % 2
        out[b, :, hh * FQ:(hh + 1) * FQ] = res.results[c]["o"]
    return out


# revision 10
# speedup vs baseline: 1.2977x; 1.0075x over previous
"""Distributed multi-head attention kernel for 8 TRN2 NeuronCores.

Problem: x[4, 2048, 1024] @ w_qkv[1024, 3072] -> qkv -> 16-head attention
         -> out[4, 2048, 1024], fp32.

Sharding (data parallel batch x tensor parallel heads):
  core c handles batch b = c // 2 and heads h0 = (c % 2) * 8 .. h0 + 8.
  Each core receives x_b [2048, 1024] and the w_qkv column slice for its
  8 heads ([1024, 1536] = q|k|v each 512 cols), produces out[b, :, 512-slice].
  All 64 (batch, head) attention problems are independent -> no collectives.

Per-core kernel (all matmuls bf16 with fp32 PSUM accumulation):
  1. x -> bf16 -> PE-transpose -> xT [c, i]       (contraction dim on partitions)
  2. qkT = w_qk.T @ x.T (per f-tile), v = x @ w_v (natural layout)
     v is stored with a fused ones-column per head: v'[:, h] = [v_h | 1]
  3. per head: dotsT[j, i] = kT.T @ qT; P = exp(dots * 0.125) (ScalarE,
     scale fused); U'[d|Z, i] += v'_h.T @ P  (PV matmul; row 64 = softmax
     denominator Z via the ones column)
  4. PE-transpose U' -> [i, d|Z]; out = U * (1/Z) (DVE); DMA out.
"""

import numpy as np

B, N, DIM = 4, 2048, 1024
HEADS, DIM_HEAD = 16, 64
INNER = HEADS * DIM_HEAD
HPC = 8                 # heads per core
FQ = HPC * DIM_HEAD     # 512 = per-core q/k/v column count
NCORES = 8

P = 128
CT = DIM // P           # 8 c-tiles (contraction)
IT = N // P             # 16 i-tiles
JT = N // P             # 16 j-tiles
NIH = 2                 # i halves
IH = N // NIH           # 1024

_CACHE = {}


def _build():
    import concourse.bass as bass
    import concourse.mybir as mybir
    import concourse.tile as tile
    from concourse import bacc
    from concourse.masks import make_identity

    f32 = mybir.dt.float32
    bf16 = mybir.dt.bfloat16
    Exp = mybir.ActivationFunctionType.Exp

    nc = bacc.Bacc(None, target_bir_lowering=False)
    x_d = nc.dram_tensor("x", [N, DIM], f32, kind="ExternalInput")
    w_d = nc.dram_tensor("w", [DIM, 3 * FQ], f32, kind="ExternalInput")
    o_d = nc.dram_tensor("o", [N, FQ], f32, kind="ExternalOutput")

    with tile.TileContext(nc) as tc, \
         tc.tile_pool(name="persist", bufs=1) as persist:
        ident = persist.tile([P, P], bf16, tag="ident", name="ident")
        make_identity(nc, ident[:])

        # persistent buffers
        xT = persist.tile([P, CT, N], bf16, tag="xT", name="xT")
        wsb = persist.tile([P, CT, 3 * FQ], bf16, tag="wsb", name="wsb")
        qkT = persist.tile([P, CT, N], bf16, tag="qkT", name="qkT")
        vp = persist.tile([P, JT, HPC * 65], bf16, tag="vp", name="vp")

        vp_heads = vp[:].rearrange("p j (h c) -> p j h c", c=65)
        nc.vector.memset(vp_heads[:, :, :, 64:65], 1.0)

        # ---- phase 1: load w first (QKV needs ALL of w but only the first
        # few x i-tiles), then x: cast + PE-transpose ----
        with tc.tile_pool(name="wload", bufs=2) as wload, \
             tc.tile_pool(name="xload", bufs=3) as xload, \
             tc.tile_pool(name="xcast", bufs=3) as xcast, \
             tc.tile_pool(name="tps_x", bufs=4, space="PSUM") as tps_x:
            for ct in range(CT):
                wf = wload.tile([P, 3 * FQ], f32)
                nc.sync.dma_start(wf[:], w_d[bass.ds(ct * P, P), :])
                nc.vector.tensor_copy(wsb[:, ct], wf[:])
            for it in range(IT):
                xf = xload.tile([P, DIM], f32)
                nc.sync.dma_start(xf[:], x_d[bass.ds(it * P, P), :])
                xb = xcast.tile([P, DIM], bf16)
                nc.vector.tensor_copy(xb[:], xf[:])
                for ct in range(CT):
                    tp = tps_x.tile([P, P], bf16)
                    nc.tensor.transpose(tp[:], xb[:, bass.ds(ct * P, P)], ident[:])
                    nc.vector.tensor_copy(xT[:, ct, bass.ds(it * P, P)], tp[:])

        # ---- phases 3+4 interleaved: QKV for head-pair hp emitted right
        # before the attention of heads 2hp, 2hp+1, so the PE fills ACT-bound
        # gaps with the next pair's QKV matmuls (keeps HAM warm, hides QKV) --
        with tc.tile_pool(name="qkvp", bufs=1, space="PSUM") as qkvp, \
             tc.tile_pool(name="dotsp", bufs=2, space="PSUM") as dotsp, \
             tc.tile_pool(name="upp", bufs=1, space="PSUM") as upp, \
             tc.tile_pool(name="tpp", bufs=1, space="PSUM") as tpp, \
             tc.tile_pool(name="ptp", bufs=3) as ptp, \
             tc.tile_pool(name="uep", bufs=2) as uep, \
             tc.tile_pool(name="recp", bufs=2) as recp, \
             tc.tile_pool(name="normp", bufs=4) as normp:

            def emit_qkv_for_pair(hp):
                # qkT f-tiles: q -> ft=hp, k -> ft=4+hp (cols hp*128..+128)
                for ft in (hp, 4 + hp):
                    for ic in range(4):
                        ps = qkvp.tile([P, 512], f32, tag="qkv")
                        for ct in range(CT):
                            nc.tensor.matmul(
                                ps[:],
                                wsb[:, ct, bass.ds(ft * P, P)],
                                xT[:, ct, bass.ds(ic * 512, 512)],
                                start=(ct == 0), stop=(ct == CT - 1),
                            )
                        nc.vector.tensor_copy(
                            qkT[:, ft, bass.ds(ic * 512, 512)], ps[:])
                # v (natural layout) for this pair: fv = hp*128 .. +128
                for it in range(IT):
                    ps = qkvp.tile([P, P], f32, tag="qkv")
                    for ct in range(CT):
                        nc.tensor.matmul(
                            ps[:],
                            xT[:, ct, bass.ds(it * P, P)],
                            wsb[:, ct, bass.ds(2 * FQ + hp * P, P)],
                            start=(ct == 0), stop=(ct == CT - 1),
                        )
                    nc.vector.tensor_copy(
                        vp_heads[:, it, bass.ds(2 * hp, 2), 0:64],
                        ps[:].rearrange("p (h c) -> p h c", c=64),
                    )

            def emit_epilogue(ups, ic, h):
                # U'[d|Z, 512] -> transpose 128-blocks -> normalize -> DMA out
                ue = uep.tile([65, 512], bf16, tag="ue")
                nc.vector.tensor_copy(ue[:], ups[:])
                for b in range(4):
                    tp = tpp.tile([P, 65], bf16, tag="tp")
                    nc.tensor.transpose(
                        tp[:], ue[:, bass.ds(b * P, P)], ident[0:65, 0:65]
                    )
                    rec = recp.tile([P, 1], f32)
                    nc.vector.reciprocal(rec[:], tp[:, 64:65])
                    nrm = normp.tile([P, 64], f32)
                    nc.vector.tensor_scalar_mul(nrm[:], tp[:, 0:64], rec[:])
                    it = ic * 4 + b
                    nc.sync.dma_start(
                        o_d[bass.ds(it * P, P), bass.ds(h * 64, 64)], nrm[:]
                    )

            def emit_attention_pair(hp):
                # heads hA = 2hp (partitions 0:64), hB = 2hp+1 (64:128) share
                # f-tiles qft/kft; dots for both packed into one [128, 1024]
                # psum (column halves) -> one exp ACTIVATE covers both.
                hA, hB = 2 * hp, 2 * hp + 1
                qft, kft = hp, 4 + hp
                for ic in range(4):          # i-chunks of 512
                    upsA = upp.tile([65, 512], f32, tag="upsA")
                    upsB = upp.tile([65, 512], f32, tag="upsB")
                    prev_pt = None
                    for j in range(JT):
                        dt_ = dotsp.tile([P, 1024], f32, tag="dt")
                        nc.tensor.matmul(
                            dt_[:, 0:512],
                            qkT[0:64, kft, bass.ds(j * P, P)],
                            qkT[0:64, qft, bass.ds(ic * 512, 512)],
                            start=True, stop=True,
                        )
                        nc.tensor.matmul(
                            dt_[:, 512:1024],
                            qkT[64:128, kft, bass.ds(j * P, P)],
                            qkT[64:128, qft, bass.ds(ic * 512, 512)],
                            start=True, stop=True,
                        )
                        pt = ptp.tile([P, 1024], bf16, tag="pt")
                        nc.scalar.activation(pt[:], dt_[:], Exp, scale=0.125)
                        if prev_pt is not None:
                            jj = j - 1
                            nc.tensor.matmul(
                                upsA[:], vp_heads[:, jj, hA, :],
                                prev_pt[:, 0:512],
                                start=(jj == 0), stop=False,
                            )
                            nc.tensor.matmul(
                                upsB[:], vp_heads[:, jj, hB, :],
                                prev_pt[:, 512:1024],
                                start=(jj == 0), stop=False,
                            )
                        prev_pt = pt
                    jj = JT - 1
                    nc.tensor.matmul(
                        upsA[:], vp_heads[:, jj, hA, :], prev_pt[:, 0:512],
                        start=False, stop=True,
                    )
                    nc.tensor.matmul(
                        upsB[:], vp_heads[:, jj, hB, :], prev_pt[:, 512:1024],
                        start=False, stop=True,
                    )
                    emit_epilogue(upsA, ic, hA)
                    emit_epilogue(upsB, ic, hB)

            for hp in range(4):
                emit_qkv_for_pair(hp)
                emit_attention_pair(hp)
    nc.finalize()
    return nc


def _get_nc():
    if "nc" not in _CACHE:
        _CACHE["nc"] = _build()
    return _CACHE["nc"]


def kernel(x: np.ndarray, w_qkv: np.ndarray) -> np.ndarray:
    from concourse.bass_utils import run_bass_kernel_spmd

    x = np.asarray(x, dtype=np.float32)
    w_qkv = np.asarray(w_qkv, dtype=np.float32)

    in_maps = []
    for c in range(NCORES):
        b, hh = c // 2, c % 2
        qo = hh * FQ
        ws = np.concatenate(
            [w_qkv[:, qo:qo + FQ],
             w_qkv[:, INNER + qo:INNER + qo + FQ],
             w_qkv[:, 2 * INNER + qo:2 * INNER + qo + FQ]], axis=1)
        in_maps.append({
            "x": np.ascontiguousarray(x[b]),
            "w": np.ascontiguousarray(ws),
        })

    nc = _get_nc()
    res = run_bass_kernel_spmd(nc, in_maps, core_ids=list(range(NCORES)))

    out = np.empty((B, N, INNER), np.float32)
    for c in range(NCORES):
        b, hh = c // 2, c % 2
        out[b, :, hh * FQ:(hh + 1) * FQ] = res.results[c]["o"]
    return out
